# revision 1
# baseline (speedup 1.0000x reference)
"""LocallyConnected2d (3x3, 64x64 out, C_in=16, C_out=32, B=32) on 8 trn2 cores.

out[b,o,h,w] = sum_{c,i,j} x[b,c,h+i,w+j] * weight[0,o,c,h,w,(i,j)] + bias[0,o,h,w]

Sharding: spatial over H_out - core i computes output rows 8i..8i+8, needing
input rows 8i..8i+10 (halo) and its 1/8 slice of the (per-position, unique)
weights. Weights dominate traffic (75MB total) and are read exactly once.

Math: per position (h,w) a K=145 contraction (9 taps x 16 ch + ones row
carrying the bias), M=32 (C_out, stationary), split K=128+17 with the K=17
part PSUM-accumulated right after (pairwise, so the accumulation dependency
is explicit in program order).

Quad packing: 4 adjacent positions share one matmul - lhsT [K, 4x32] and
rhs [K, 4x32] produce a [128, 128] PSUM block whose 32x32 diagonal blocks
are the 4 positions' [C_out, B] outputs (off-diagonal blocks are discarded).
This quarters the PE instruction count; the diagonal extraction is free:
the per-col-group out-DMA just slices partitions 32j..32j+32 x cols
32j..32j+32.

The host pre-replicates x into a [145, T, B] "shifted windows" tensor (one
16-channel band per kernel tap (i,j), shifted by 66*i+j, plus the ones row),
so the moving operand of quad (h, w0) is xr[:, 66h+w0 : 66h+w0+4, :] -
contiguous, no im2col on device.
"""

import numpy as np

import concourse.bass as bass
import concourse.mybir as mybir
import concourse.tile as tile
from concourse import bacc
from concourse import bass_utils

N_CORES = 8
B, CI, CO = 32, 16, 32
H = W = 64
HL = H // N_CORES          # output rows per core
XROWS = HL + 2             # input rows per core (with halo)
XW = 66
XFLAT = XROWS * XW         # 660
T = HL * XW                # 528: padded flat window (8 chunks of 66)
KA, KB = 128, 17           # contraction split
KT = KA + KB               # 145
NQ = W // 4                # 16 quads per row

USE_BF16 = True

_cache = {}


def _np_dt(use_bf16):
    if use_bf16:
        import ml_dtypes
        return np.dtype(ml_dtypes.bfloat16)
    return np.dtype(np.float32)


def _build(use_bf16, n_iters=1, mode="full"):
    assert mode in ("full", "lag", "dma", "xdma", "pe", "pelag")
    do_pe = mode not in ("dma", "xdma")
    pe_only = mode in ("pe", "pelag", "xdma")
    lag = mode in ("lag", "pelag")
    dt = mybir.dt.bfloat16 if use_bf16 else mybir.dt.float32
    f32 = mybir.dt.float32
    nc = bacc.Bacc("TRN2", target_bir_lowering=False, debug=False,
                   num_devices=N_CORES)
    xr_d = nc.dram_tensor("xr", [KT, T, B], dt, kind="ExternalInput")
    wsa_d = nc.dram_tensor("wsa", [HL, KA, W, CO], dt, kind="ExternalInput")
    wsb_d = nc.dram_tensor("wsb", [KB, HL, W, CO], dt, kind="ExternalInput")
    out_d = nc.dram_tensor("out", [HL, 4, CO, NQ, B], f32,
                           kind="ExternalOutput")

    import contextlib

    with tile.TileContext(nc) as tc:
        with (
            tc.tile_pool(name="px", bufs=1) as px,
            tc.tile_pool(name="pwa", bufs=4) as pwa,
            tc.tile_pool(name="pwb", bufs=2) as pwb,
            tc.tile_pool(name="po", bufs=2) as po,
            tc.tile_pool(name="pp", bufs=4, space=bass.MemorySpace.PSUM) as pp,
        ):
            loop = (tc.For_i(0, n_iters, 1) if n_iters > 1
                    else contextlib.nullcontext())
            with loop:
                pa = px.tile([KA, T, B], dt, tag="pa")
                pb = px.tile([KB, T, B], dt, tag="pb")
                for h in range(HL):
                    sl = slice(XW * h, XW * (h + 1))
                    nc.sync.dma_start(pa[:, sl, :], xr_d[0:KA, sl, :])
                nc.scalar.dma_start(pb[:], xr_d[KA:KT, :, :])

                wb_all = None
                if use_bf16:
                    wb_all = pwb.tile([KB, HL, W, CO], dt, tag="wb")
                    nc.scalar.dma_start(wb_all[:], wsb_d[:])

                if pe_only and mode != "xdma":
                    wa0 = pwa.tile([KA, W, CO], dt, tag="wa")
                    nc.scalar.dma_start(wa0[:], wsa_d[0])

                for h in range(HL):
                    weng = nc.sync if h % 2 == 0 else nc.scalar
                    stage = po.tile([KA, NQ, KA], f32, tag="stage")
                    if use_bf16:
                        wbh = wb_all[:, h] if wb_all is not None else None
                    else:
                        wbh = pwb.tile([KB, W, CO], dt, tag="wbh")
                        nc.scalar.dma_start(wbh[:], wsb_d[:, h])
                    if mode in ("dma", "xdma") and (h == 0 or mode == "dma"):
                        nc.gpsimd.memset(stage[:], 0.0)
                    if mode == "xdma":
                        wa = None
                    elif pe_only:
                        wa = wa0
                    else:
                        wa = pwa.tile([KA, W, CO], dt, tag="wa")
                        weng.dma_start(wa[:], wsa_d[h])
                    if do_pe and lag:
                        # lag-2 interleave: B(q) issues two MMs after A(q),
                        # hiding A's PSUM drain behind A(q+1)'s fill.
                        tiles, mma, mmb = [], [], []
                        for q in range(NQ):
                            w0 = 4 * q
                            t0 = XW * h + w0
                            ps = pp.tile([KA, KA], f32, tag="ps")
                            tiles.append(ps)
                            mma.append((ps, wa[:, w0:w0 + 4, :],
                                        pa[:, t0:t0 + 4, :]))
                            mmb.append((ps, wbh[:, w0:w0 + 4, :],
                                        pb[:, t0:t0 + 4, :]))
                        sched = []
                        for q in range(NQ):
                            sched.append(("a", q))
                            if q >= 1:
                                sched.append(("b", q - 1))
                        sched.append(("b", NQ - 1))
                        for kind, q in sched:
                            ps, lhs, rhs = mma[q] if kind == "a" else mmb[q]
                            nc.tensor.matmul(ps[:], lhs, rhs,
                                             start=(kind == "a"),
                                             stop=(kind == "b"))
                            if kind == "b":
                                nc.vector.tensor_copy(stage[:, q, :], ps[:])
                    elif do_pe:
                        # independent A/B quads in bank-grouped PSUM; two DVE
                        # passes (copy + accumulate) combine them.
                        for gq in range(NQ // 4):
                            psa = pp.tile([KA, 4, KA], f32, tag="psa")
                            psb = pp.tile([KA, 4, KA], f32, tag="psb")
                            for qq in range(4):
                                q = 4 * gq + qq
                                w0 = 4 * q
                                t0 = XW * h + w0
                                nc.tensor.matmul(psa[:, qq, :],
                                                 wa[:, w0:w0 + 4, :],
                                                 pa[:, t0:t0 + 4, :],
                                                 start=True, stop=True)
                            for qq in range(4):
                                q = 4 * gq + qq
                                w0 = 4 * q
                                t0 = XW * h + w0
                                nc.tensor.matmul(psb[:, qq, :],
                                                 wbh[:, w0:w0 + 4, :],
                                                 pb[:, t0:t0 + 4, :],
                                                 start=True, stop=True)
                            ssl = stage[:, 4 * gq:4 * (gq + 1), :]
                            nc.vector.tensor_copy(ssl, psa[:])
                            nc.vector.tensor_add(ssl, ssl, psb[:])
                    if not pe_only or h == 0:
                        for j in range(4):
                            nc.sync.dma_start(
                                out_d[h, j],
                                stage[32 * j:32 * (j + 1), :,
                                      32 * j:32 * (j + 1)])
    nc.compile()
    return nc


def _get_nc(use_bf16, n_iters=1, mode="full"):
    key = (use_bf16, n_iters, mode)
    if key not in _cache:
        _cache[key] = _build(use_bf16, n_iters, mode)
    return _cache[key]


def _pack_inputs(x, weight, bias, use_bf16):
    """Full inputs -> per-core in_maps (host-side shard + relayout)."""
    np_dt = _np_dt(use_bf16)
    x = np.asarray(x, np.float32)
    weight = np.asarray(weight, np.float32)
    bias = np.asarray(bias, np.float32)

    # weights: [1,o,c,h,w,k] -> [h, w, k=(tap,kc), o], bias appended as k=144
    wt = weight[0].transpose(2, 3, 4, 1, 0).reshape(H, W, 9 * CI, CO)
    bt = bias[0].transpose(1, 2, 0)[:, :, None, :]          # [h, w, 1, o]
    wfull = np.concatenate([wt, bt], axis=2)                # [h, w, 145, o]

    in_maps = []
    for c in range(N_CORES):
        r0 = HL * c
        xs = x[:, :, r0:r0 + XROWS, :].transpose(1, 0, 2, 3).reshape(
            CI, B, XFLAT)                                   # [c, b, flat]
        xr = np.zeros((KT, T, B), np.float32)
        for k in range(9):
            i, j = divmod(k, 3)
            off = XW * i + j
            blk = xs[:, :, off:off + T - 2]                 # [16, 32, 526]
            xr[16 * k:16 * (k + 1), :T - 2, :] = blk.transpose(0, 2, 1)
        xr[144, :, :] = 1.0
        in_maps.append({"xr": np.ascontiguousarray(xr, dtype=np_dt)})

        wc = wfull[r0:r0 + HL].transpose(0, 2, 1, 3)        # [h, k, w, o]
        in_maps[-1]["wsa"] = np.ascontiguousarray(wc[:, :KA], dtype=np_dt)
        in_maps[-1]["wsb"] = np.ascontiguousarray(
            wc[:, KA:].transpose(1, 0, 2, 3), dtype=np_dt)  # [kb, h, w, o]
    return in_maps


def _gather(results):
    # per-core out: [HL, 4(j), CO, NQ(q), B]; w = 4q + j
    outs = np.stack([results[c]["out"] for c in range(N_CORES)])
    out = outs.transpose(5, 3, 0, 1, 4, 2)     # [b, o, core, h, q, j]
    out = out.reshape(B, CO, H, W)
    return np.ascontiguousarray(out)


def run(x, weight, bias, use_bf16=None, n_iters=1, mode="full", **spmd_kwargs):
    if use_bf16 is None:
        use_bf16 = USE_BF16
    nc = _get_nc(use_bf16, n_iters, mode)
    in_maps = _pack_inputs(x, weight, bias, use_bf16)
    res = bass_utils.run_bass_kernel_spmd(nc, in_maps,
                                          core_ids=list(range(N_CORES)),
                                          **spmd_kwargs)
    return _gather(res.results), res


def kernel(x, weight, bias):
    out, _ = run(x, weight, bias)
    return out

